# revision 12
# baseline (speedup 1.0000x reference)
"""Fused attention block (qkv proj + pooled attention + 16-head masked
attention + out proj) for TRN2, batch-parallel across 8 NeuronCores.

Layouts per core (batch element b):
  xT      [D=1024, n=1024]   x transposed (host-side), feature-major
  qT,kT   [f, i]  feature-major  (8 tiles each [128, 1024])
  v'      [j, 65*16]  position-major, per-head 64 cols + a ones column
  P^T     [j, i]  unnormalized exp(attention) transposed
  aoT     [f, i]  un/normalized head outputs, feature-major
Outputs: out [n, D] natural, attn_ [n, n] natural.

Masked softmax without max-subtraction (args are small, exp safe):
  key mask  -> additive -80 bias on exp (per j-tile partition bias)
  query mask-> rows fixed at the out-projection via a rank-2 correction
               matmul adding  ones*b_out + (1-qkeep)*ybar  after zeroing
               masked columns through the normalization factor.
"""
import os
import sys

sys.path.insert(0, "/opt/trn_rl_repo")

import numpy as np

import concourse.bass as bass
import concourse.mybir as mybir
import concourse.tile as tile
from concourse import bacc, bass_utils

F32 = mybir.dt.float32
F32R = mybir.dt.float32r
EXP = mybir.ActivationFunctionType.Exp

B = 8
N = 1024          # sequence (after CLS pad)
D = 1024          # model dim
H = 16
DH = 64
NT = N // 128     # 8 row tiles
SCALE_H = DH ** -0.5     # 1/8
SCALE_P = D ** -0.5      # 1/32
NEG = -80.0

_CACHED = {}


def build_nc():
    nc = bacc.Bacc("TRN2", target_bir_lowering=False, debug=False, num_devices=8)
    xT = nc.dram_tensor("xT", [D, N], F32R, kind="ExternalInput").ap()
    wqkp = nc.dram_tensor("wqkp", [D, 2 * D], F32R, kind="ExternalInput").ap()
    wv_in = nc.dram_tensor("wv_in", [D, D], F32R, kind="ExternalInput").ap()
    wout = nc.dram_tensor("wout", [D, D], F32R, kind="ExternalInput").ap()
    bout = nc.dram_tensor("bout", [1, D], F32R, kind="ExternalInput").ap()
    cmaskT = nc.dram_tensor("cmaskT", [128, NT], F32, kind="ExternalInput").ap()
    qkeep16 = nc.dram_tensor("qkeep16", [H, N], F32, kind="ExternalInput").ap()
    fixl_in = nc.dram_tensor("fixl_in", [2, N], F32R, kind="ExternalInput").ap()
    sel_in = nc.dram_tensor("sel_in", [H, D], F32R, kind="ExternalInput").ap()
    vones_in = nc.dram_tensor("vones_in", [128, H], F32R, kind="ExternalInput").ap()
    out_d = nc.dram_tensor("out", [N, D], F32, kind="ExternalOutput").ap()
    attn_d = nc.dram_tensor("attn", [N, N], F32, kind="ExternalOutput").ap()

    with tile.TileContext(nc, trace_sim=bool(os.environ.get('ATTN_TRACE_SIM'))) as tc:
        with (
            tc.tile_pool(name="big8", bufs=8) as big8,      # xT then aoT (slot reuse)
            tc.tile_pool(name="qk", bufs=16) as qkp,
            tc.tile_pool(name="vp", bufs=8) as vpp,
            tc.tile_pool(name="pt", bufs=3) as ptp,
            tc.tile_pool(name="wq", bufs=3) as wqp,
            tc.tile_pool(name="wv", bufs=3) as wvp,
            tc.tile_pool(name="wo", bufs=2) as wop,
            tc.tile_pool(name="outt", bufs=2) as outp,
            tc.tile_pool(name="one", bufs=1) as onep,
            tc.tile_pool(name="small", bufs=4) as smallp,
            tc.tile_pool(name="mm", bufs=2, space="PSUM") as mmp,
            tc.tile_pool(name="av", bufs=2, space="PSUM") as avp,
        ):
            # ---- constants ----
            cmask_t = onep.tile([128, NT], F32, name="cmask_t", tag="cmask_t")
            nc.gpsimd.dma_start(out=cmask_t, in_=cmaskT)
            qkeep_t = onep.tile([H, N], F32, name="qkeep_t", tag="qkeep_t")
            nc.gpsimd.dma_start(out=qkeep_t, in_=qkeep16)
            fixl1 = onep.tile([1, N], F32R, name="fixl1", tag="fixl1")
            nc.gpsimd.dma_start(out=fixl1, in_=fixl_in[0:1, :])
            fixl2 = onep.tile([1, N], F32R, name="fixl2", tag="fixl2")
            nc.gpsimd.dma_start(out=fixl2, in_=fixl_in[1:2, :])
            sel = onep.tile([H, D], F32R, name="sel", tag="sel")
            nc.gpsimd.dma_start(out=sel, in_=sel_in)
            fixr1 = onep.tile([1, N], F32R, name="fixr1", tag="fixr1")
            nc.gpsimd.dma_start(out=fixr1, in_=bout)
            fixr2 = onep.tile([1, N], F32R, name="fixr2", tag="fixr2")
            onescol = onep.tile([128, 1], F32, name="onescol", tag="onescol")
            nc.vector.memset(onescol, 1.0 / N)
            vbarT = onep.tile([128, NT], F32R, name="vbarT", tag="vbarT")
            srecraw = onep.tile([H, N], F32, name="srecraw", tag="srecraw")
            srec = onep.tile([H, N], F32, name="srec", tag="srec")
            sq = onep.tile([H, N], F32R, name="sq", tag="sq")

            # ---- load xT ----
            xts = []
            for t in range(NT):
                xt = big8.tile([128, N], F32R, tag="big", name=f"xt{t}")
                nc.sync.dma_start(out=xt, in_=xT[t * 128:(t + 1) * 128, :])
                xts.append(xt)

            # ---- V position-major (v' with per-head ones column) ----
            v_tiles = []
            for t in range(NT):
                vt = vpp.tile([128, 65 * H], F32R, tag="v", name=f"v{t}")
                v3 = vt.rearrange("p (h d) -> p h d", d=65)
                nc.gpsimd.dma_start(out=v3[:, :, 64:65], in_=vones_in[:, :, None])
                v_tiles.append(vt)
            for pass_ in range(2):
                pss = []
                for q4 in range(4):
                    pool = mmp if q4 < 2 else avp
                    ps = pool.tile([128, N], F32, tag="mm" if q4 < 2 else "av",
                                   name=f"ps_v{pass_}{q4}")
                    pss.append(ps)
                for kt in range(NT):
                    for c in range(2):
                        w = wvp.tile([128, 512], F32R, tag="wv", name="w_v")
                        nc.sync.dma_start(
                            out=w,
                            in_=wv_in[kt * 128:(kt + 1) * 128,
                                      c * 512:(c + 1) * 512],
                        )
                        for q4 in range(4):
                            it = pass_ * 4 + q4
                            nc.tensor.matmul(
                                pss[q4][:, c * 512:(c + 1) * 512],
                                xts[kt][:, it * 128:(it + 1) * 128],
                                w[:],
                                start=(kt == 0),
                                stop=(kt == NT - 1),
                            )
                for q4 in range(4):
                    it = pass_ * 4 + q4
                    v3 = v_tiles[it].rearrange("p (h d) -> p h d", d=65)
                    for c in range(2):
                        src = pss[q4][:, c * 512:(c + 1) * 512].rearrange(
                            "p (h d) -> p h d", d=64)
                        nc.vector.tensor_copy(v3[:, c * 8:(c + 1) * 8, 0:64], src)

            # ---- QKV: qT,kT feature-major (head-pair emission order) ----
            qk_tiles = [None] * (2 * NT)
            for fpair in range(NT):
                pair_ps = []
                for half in (0, 1):
                    ft = fpair + half * NT
                    pool = mmp if half == 0 else avp
                    ps = pool.tile([128, N], F32, tag="mm" if half == 0 else "av",
                                   name=f"ps_qk{ft}")
                    pair_ps.append(ps)
                for kt in range(NT):
                    w = wqp.tile([128, 256], F32R, tag="wq", name="w_qk")
                    nc.sync.dma_start(
                        out=w,
                        in_=wqkp[kt * 128:(kt + 1) * 128,
                                 fpair * 256:(fpair + 1) * 256])
                    for half in (0, 1):
                        for c in range(2):
                            nc.tensor.matmul(
                                pair_ps[half][:, c * 512:(c + 1) * 512],
                                w[:, half * 128:(half + 1) * 128],
                                xts[kt][:, c * 512:(c + 1) * 512],
                                start=(kt == 0),
                                stop=(kt == NT - 1),
                            )
                for half in (0, 1):
                    ft = fpair + half * NT
                    qk = qkp.tile([128, N], F32R, tag="qk", name=f"qk{ft}")
                    nc.vector.tensor_copy(qk, pair_ps[half])
                    qk_tiles[ft] = qk

                # vbar f-tile for this pair (fills DMA gaps; needs only v')
                t = fpair
                psv = mmp.tile([128, N], F32, tag="mm", name=f"ps_vb{t}")
                for jt in range(NT):
                    for u in range(2):
                        h = 2 * t + u
                        nc.tensor.matmul(
                            psv[u * 64:(u + 1) * 64, 0:1],
                            v_tiles[jt][:, h * 65:h * 65 + 64].bitcast(F32),
                            onescol[:],
                            start=(jt == 0), stop=(jt == NT - 1),
                        )
                nc.vector.tensor_copy(vbarT[:, t:t + 1], psv[:, 0:1])

            # ---- heads (paired for row-tiling concurrency) + pooled ----
            ao_tiles = []
            for ft in range(NT):
                ao = big8.tile([128, N], F32R, tag="big", name=f"ao{ft}")
                ao_tiles.append(ao)

            for hp in range(NT):  # head pair index = f-tile of qk
                avs = []
                for u in range(2):
                    a = avp.tile([128, N], F32, tag="av", name=f"av{hp}{u}")
                    avs.append(a)
                for jt in range(NT):
                    dps = []
                    for u in range(2):
                        h = 2 * hp + u
                        off = u * 64
                        dp = mmp.tile([128, N], F32, tag="mm", name=f"dp{h}{jt}")
                        kt_ap = qk_tiles[NT + hp][off:off + 64,
                                                  jt * 128:(jt + 1) * 128]
                        for c in range(2):
                            qt_ap = qk_tiles[hp][off:off + 64,
                                                 c * 512:(c + 1) * 512]
                            nc.tensor.matmul(
                                dp[:, c * 512:(c + 1) * 512], kt_ap, qt_ap,
                                start=True, stop=True,
                            )
                        dps.append(dp)
                    for u in range(2):
                        h = 2 * hp + u
                        pt = ptp.tile([128, N], F32R, tag="pt", name=f"pt{h}{jt}")
                        nc.scalar.activation(
                            pt, dps[u], EXP,
                            bias=cmask_t[:, jt:jt + 1], scale=SCALE_H,
                        )
                        vh = v_tiles[jt][:, h * 65:(h + 1) * 65]
                        for c in range(2):
                            nc.tensor.matmul(
                                avs[u][0:65, c * 512:(c + 1) * 512],
                                vh, pt[:, c * 512:(c + 1) * 512],
                                start=(jt == 0), stop=(jt == NT - 1),
                            )
                for u in range(2):
                    h = 2 * hp + u
                    nc.vector.tensor_copy(
                        ao_tiles[hp][u * 64:(u + 1) * 64, :], avs[u][0:64, :])
                    stg = smallp.tile([1, N], F32, tag="stg", bufs=1, name=f"stg{h}")
                    nc.vector.tensor_copy(stg, avs[u][64:65, :])
                    nc.gpsimd.dma_start(out=srecraw[h:h + 1, :], in_=stg)

                # pooled attention tile hp (interleaved for ACT/PE overlap)
                it = hp
                ps = mmp.tile([128, N], F32, tag="mm", name=f"ps_pool{it}")
                for ft in range(NT):
                    lhs = qk_tiles[ft][:, it * 128:(it + 1) * 128]
                    for c in range(2):
                        nc.tensor.matmul(
                            ps[:, c * 512:(c + 1) * 512],
                            lhs, qk_tiles[NT + ft][:, c * 512:(c + 1) * 512],
                            start=(ft == 0), stop=(ft == NT - 1),
                        )
                pe = ptp.tile([128, N], F32, tag="pt", name=f"pe{it}")
                sums = smallp.tile([128, 1], F32, tag="sm", name=f"sums{it}")
                nc.scalar.activation(pe, ps, EXP, scale=SCALE_P, accum_out=sums)
                rec = smallp.tile([128, 1], F32, tag="rc", name=f"rec{it}")
                nc.vector.reciprocal(rec, sums)
                ot = outp.tile([128, N], F32, tag="ot", name=f"ot_a{it}")
                nc.vector.tensor_scalar_mul(ot, pe, rec)
                nc.gpsimd.dma_start(out=attn_d[it * 128:(it + 1) * 128, :], in_=ot)

            # ---- normalize head outputs (zero masked-query columns) ----
            nc.vector.reciprocal(srec[:], srecraw[:])
            nc.vector.tensor_mul(sq[:], srec[:], qkeep_t[:])
            for ft in range(NT):
                bp = mmp.tile([128, N], F32, tag="mm", name=f"bp{ft}")
                for c in range(2):
                    nc.tensor.matmul(
                        bp[:, c * 512:(c + 1) * 512],
                        sel[:, ft * 128:(ft + 1) * 128],
                        sq[:, c * 512:(c + 1) * 512],
                        start=True, stop=True,
                    )
                nc.vector.tensor_mul(ao_tiles[ft], ao_tiles[ft], bp)

            # ---- ybar = vbar @ wout (row for masked queries) ----
            ybps = mmp.tile([128, N], F32, tag="mm", name="ybps")
            for ft in range(NT):
                w = wop.tile([128, N], F32R, tag="wo", name="w_y")
                nc.gpsimd.dma_start(out=w, in_=wout[ft * 128:(ft + 1) * 128, :])
                for c in range(2):
                    nc.tensor.matmul(
                        ybps[0:1, c * 512:(c + 1) * 512],
                        vbarT[:, ft:ft + 1],
                        w[:, c * 512:(c + 1) * 512],
                        start=(ft == 0), stop=(ft == NT - 1),
                    )
            nc.vector.tensor_copy(fixr2, ybps[0:1, :])

            # ---- out projection (+rank-2 fix) ----
            for pass_ in range(2):
                pss = []
                for q4 in range(4):
                    pool = mmp if q4 < 2 else avp
                    ps = pool.tile([128, N], F32, tag="mm" if q4 < 2 else "av",
                                   name=f"ps_o{pass_}{q4}")
                    pss.append(ps)
                for ft in range(NT):
                    w = wop.tile([128, N], F32R, tag="wo", name="w_o")
                    nc.gpsimd.dma_start(
                        out=w, in_=wout[ft * 128:(ft + 1) * 128, :])
                    for c in range(2):
                        for q4 in range(4):
                            it = pass_ * 4 + q4
                            nc.tensor.matmul(
                                pss[q4][:, c * 512:(c + 1) * 512],
                                ao_tiles[ft][:, it * 128:(it + 1) * 128],
                                w[:, c * 512:(c + 1) * 512],
                                start=(ft == 0), stop=False,
                            )
                for q4 in range(4):
                    it = pass_ * 4 + q4
                    for c in range(2):
                        nc.tensor.matmul(
                            pss[q4][:, c * 512:(c + 1) * 512],
                            fixl1[:, it * 128:(it + 1) * 128],
                            fixr1[:, c * 512:(c + 1) * 512],
                            start=False, stop=False,
                        )
                        nc.tensor.matmul(
                            pss[q4][:, c * 512:(c + 1) * 512],
                            fixl2[:, it * 128:(it + 1) * 128],
                            fixr2[:, c * 512:(c + 1) * 512],
                            start=False, stop=True,
                        )
                    ot = outp.tile([128, N], F32, tag="ot", name=f"ot_o{it}")
                    nc.vector.tensor_copy(ot, pss[q4])
                    nc.sync.dma_start(
                        out=out_d[it * 128:(it + 1) * 128, :], in_=ot)

    nc.compile()
    return nc


def _host_prep(x, mask, w_qkv, w_out, b_out):
    in_maps = []
    sel = np.zeros((H, D), dtype=np.float32)
    sel[np.arange(D) // DH, np.arange(D)] = 1.0
    wq3 = w_qkv.reshape(D, 3, NT, 128)
    wqkp = np.ascontiguousarray(
        np.stack([wq3[:, 0], wq3[:, 1]], axis=2).reshape(D, 2 * D))
    wv = np.ascontiguousarray(w_qkv[:, 2 * D:])
    for b in range(B):
        m = np.concatenate([[True], mask[b]]).astype(np.float32)  # [N]
        cm = np.where(m > 0, 0.0, NEG).astype(np.float32)
        in_maps.append({
            "xT": np.ascontiguousarray(x[b].T),
            "wqkp": wqkp,
            "wv_in": wv,
            "wout": w_out,
            "bout": b_out.reshape(1, D),
            "cmaskT": np.ascontiguousarray(cm.reshape(NT, 128).T),
            "qkeep16": np.tile(m, (H, 1)),
            "fixl_in": np.stack([np.ones(N, np.float32), 1.0 - m]),
            "sel_in": sel,
            "vones_in": np.ones((128, H), np.float32),
        })
    return in_maps


def kernel(x, mask, w_qkv, w_out, b_out, **run_kw):
    if "nc" not in _CACHED:
        _CACHED["nc"] = build_nc()
    nc = _CACHED["nc"]
    in_maps = _host_prep(
        np.asarray(x, np.float32), np.asarray(mask),
        np.asarray(w_qkv, np.float32), np.asarray(w_out, np.float32),
        np.asarray(b_out, np.float32))
    res = bass_utils.run_bass_kernel_spmd(
        nc, in_maps, core_ids=list(range(B)), **run_kw)
    out = np.stack([res.results[b]["out"] for b in range(B)])
    attn_ = np.stack([res.results[b]["attn"] for b in range(B)])
    _CACHED["last_results"] = res
    return out, attn_
